# revision 15
# baseline (speedup 1.0000x reference)
"""Trainium2 Bass kernel for the gnn_message_passing reward environment.

reference:
    diff   = feature - next_feature                    # [N, D]
    neigh  = next_action @ diff                        # [N, D]
    impact = (neigh @ neigh.T) / D                     # [N, N]
    normed = row_l2_normalize(next_feature)            # [N, D]
    sim    = normed @ normed.T                         # [N, N]
    out    = persona_a * next_action * sim             # reward_sim
           - persona_b * edges                         # reward_cost
           + persona_g * impact                        # reward_impact
    (persona_x = persona_t @ x, per-row scalars)

Distribution: 1D row shard across 8 NeuronCores (512 rows each).
All three GEMMs run in fp8e4m3 with DoubleRow perf mode (2x fp8 rate at
FD=512). diff and normed.T are precomputed host-side and staged as fp8
inputs (replicated), so phase 1 (neigh.T = diff.T @ A_shard.T) and
phase 2 (sim row-shard) have no collective dependency; the single
AllGather shares the fp8 neigh.T shards for phase 3 (impact GEMM) and
overlaps phase 2 plus the NRT rank barrier. Elementwise reward combine
is fused on DVE reading straight out of PSUM; output is written bf16.
"""
import numpy as np
import ml_dtypes
from contextlib import ExitStack

import concourse.bass as bass
import concourse.tile as tile
from concourse import bacc, mybir
from concourse.bass_utils import run_bass_kernel_spmd

N = 4096          # graph nodes
D = 1024          # feature dim
NPERS = 8         # personas
NCORES = 8
R = N // NCORES   # 512 rows per core
RT = R // 128     # 4 row tiles per shard
DT = D // 128     # 8 d-tiles
K2 = N // 256     # 16 contraction k-tile PAIRS for A @ diff (DoubleRow)
D2 = DT // 2      # 4 contraction pairs over D
NB = N // 512     # 8 output column blocks

F32 = mybir.dt.float32
BF16 = mybir.dt.bfloat16
F8 = mybir.dt.float8e4
DRow = mybir.MatmulPerfMode.DoubleRow
MUL = mybir.AluOpType.mult
ADD = mybir.AluOpType.add


def build(reps: int = 1, stage: int = 4, mock_cc: bool = False):
    nc = bacc.Bacc("TRN2", target_bir_lowering=False, debug=False,
                   num_devices=NCORES)

    diff8 = nc.dram_tensor("diff8", [N, D], F8, kind="ExternalInput").ap()
    at8 = nc.dram_tensor("at8", [N, R], F8, kind="ExternalInput").ap()
    nto8 = nc.dram_tensor("nto8", [D, R], F8, kind="ExternalInput").ap()
    ntr8 = nc.dram_tensor("ntr8", [D, N], F8, kind="ExternalInput").ap()
    am8 = nc.dram_tensor("am8", [R, N], F8, kind="ExternalInput").ap()
    ed8 = nc.dram_tensor("ed8", [R, N], F8, kind="ExternalInput").ap()
    pt = nc.dram_tensor("pt", [NPERS, R], F32, kind="ExternalInput").ap()
    gmat = nc.dram_tensor("gmat", [NPERS, 3], F32, kind="ExternalInput").ap()
    out = nc.dram_tensor("out", [R, N], BF16, kind="ExternalOutput").ap()

    rgroups = [list(range(NCORES))]

    def blk(ap):
        """[T*128, M] -> [128, T, M] partition-tiled view."""
        return ap.rearrange("(a p) m -> p a m", p=128)

    with tile.TileContext(nc) as tc, ExitStack() as ctx:
        const = ctx.enter_context(tc.tile_pool(name="const", bufs=1))
        big = ctx.enter_context(tc.tile_pool(name="big", bufs=1))
        own = ctx.enter_context(tc.tile_pool(name="own", bufs=1))
        stream = ctx.enter_context(tc.tile_pool(name="stream", bufs=1))
        outp_pool = ctx.enter_context(tc.tile_pool(name="outp", bufs=1))
        ps = ctx.enter_context(tc.tile_pool(name="ps", bufs=8, space="PSUM"))
        dram = ctx.enter_context(tc.tile_pool(name="dram", bufs=1, space="DRAM"))

        pt_sb = const.tile([NPERS, R], F32)
        nc.sync.dma_start(pt_sb[:], pt[:])
        gmat_sb = const.tile([NPERS, 3], F32)
        nc.sync.dma_start(gmat_sb[:], gmat[:])

        # phase-2 SBUF-resident operands. nto is tiny and loads up front;
        # ntr interleaves with the phase-1 streams (d8 piece per k2 pair)
        # and am/ed issue after phase 1 so its streams aren't queue-blocked.
        nto_sb = big.tile([128, DT, R], F8, name="nto_sb", tag="nto")
        nc.sync.dma_start(nto_sb[:], blk(nto8))
        ntrch = [big.tile([128, 2, N], F8, name=f"ntrch{k2}", tag=f"ntrch{k2}")
                 for k2 in range(D2)]
        am_sb = big.tile([128, RT, N], F8, name="am_sb", tag="am")
        ed_sb = big.tile([128, RT, N], F8, name="ed_sb", tag="ed")

        for rep in range(reps):
            # ---------------- phase 0: persona scalars ----------------
            # pa=alpha-mix/256 (16x-scaled normed), pbn=-beta-mix, pgs=gamma-mix/D
            pa_sb = const.tile([128, RT], F32, name=f"pa_sb{rep}", tag="pa")
            pbn_sb = const.tile([128, RT], F32, name=f"pbn_sb{rep}", tag="pbn")
            pgs_sb = const.tile([128, RT], F32, name=f"pgs_sb{rep}", tag="pgs")
            for mt in range(RT):
                pp = ps.tile([128, 512], F32, name=f"pp{rep}_{mt}", tag="ps")
                nc.tensor.matmul(pp[:, 0:3], pt_sb[:, mt * 128:(mt + 1) * 128],
                                 gmat_sb[:], start=True, stop=True)
                nc.scalar.mul(pa_sb[:, mt:mt + 1], pp[:, 0:1], 1.0 / 256)
                nc.scalar.mul(pbn_sb[:, mt:mt + 1], pp[:, 1:2], -1.0)
                nc.scalar.mul(pgs_sb[:, mt:mt + 1], pp[:, 2:3], 1.0 / D)

            ag_ne_in = dram.tile([D, R], F8, name=f"ag_ne_in{rep}", tag="agei")
            ag_ne_out = dram.tile([NCORES, D, R], F8, addr_space="Shared",
                                  name=f"ag_ne_out{rep}", tag="ageo")

            # ---------------- phase 1: neigh.T = diff.T @ A_shard.T ----------
            g1ps = []
            for d8 in range(DT):
                t = ps.tile([128, 512], F32, name=f"g1ps{rep}_{d8}", tag="ps")
                g1ps.append(t)
            neighT_own = own.tile([128, DT, R], F8,
                                  name=f"neown{rep}", tag="neown")
            for k2 in range(K2):
                dch = stream.tile([128, 2, D], F8, name=f"dch{rep}_{k2}",
                                  tag="dch", bufs=6)
                nc.sync.dma_start(dch[:], blk(diff8)[:, 2 * k2:2 * k2 + 2, :])
                ach = stream.tile([128, 2, R], F8, name=f"ach{rep}_{k2}",
                                  tag="ach", bufs=6)
                nc.sync.dma_start(ach[:], blk(at8)[:, 2 * k2:2 * k2 + 2, :])
                if rep == 0 and k2 % 4 == 3:
                    kp = k2 // 4
                    nc.sync.dma_start(ntrch[kp][:],
                                      blk(ntr8)[:, 2 * kp:2 * kp + 2, :])
                if k2 < K2 - 1:
                    for d8 in range(DT):
                        nc.tensor.matmul(
                            g1ps[d8][:],
                            dch[:, :, d8 * 128:(d8 + 1) * 128],
                            ach[:], start=(k2 == 0), stop=False,
                            perf_mode=DRow)
                else:
                    # finish banks one at a time; drain (Scalar+Vector
                    # alternating) + AG-input write pipeline under the
                    # remaining MMs
                    for d8 in range(DT):
                        nc.tensor.matmul(
                            g1ps[d8][:],
                            dch[:, :, d8 * 128:(d8 + 1) * 128],
                            ach[:], start=False, stop=True,
                            perf_mode=DRow)
                        if d8 % 2 == 0:
                            nc.scalar.copy(neighT_own[:, d8, :], g1ps[d8][:])
                        else:
                            nc.vector.tensor_copy(neighT_own[:, d8, :],
                                                  g1ps[d8][:])
                        nc.sync.dma_start(
                            ag_ne_in[d8 * 128:(d8 + 1) * 128, :],
                            neighT_own[:, d8, :])

            if mock_cc:
                nc.sync.dma_start(ag_ne_out[0][:], ag_ne_in[:])
            else:
                nc.gpsimd.collective_compute(
                    "AllGather", mybir.AluOpType.bypass, ins=[ag_ne_in.opt()],
                    outs=[ag_ne_out.opt()], replica_groups=rgroups)

            if rep == 0:
                nc.sync.dma_start(am_sb[:], blk(am8))
                nc.sync.dma_start(ed_sb[:], blk(ed8))

            if stage <= 1:
                for d8 in range(DT):
                    nc.gpsimd.dma_start(out[0:128, d8 * 512:(d8 + 1) * 512],
                                        neighT_own[:, d8, :])
                continue

            # ---------------- phase 2: sim GEMM + mask*alpha - edges*beta ----
            outp = outp_pool.tile([128, RT, N], BF16, name=f"outp{rep}",
                                  tag="outp")
            for nb in range(NB):
                csl = slice(nb * 512, (nb + 1) * 512)
                for mt in range(RT):
                    sps = ps.tile([128, 512], F32, name=f"sps{rep}_{nb}_{mt}",
                                  tag="ps")
                    for k2 in range(D2):
                        nc.tensor.matmul(
                            sps[:],
                            nto_sb[:, 2 * k2:2 * k2 + 2,
                                   mt * 128:(mt + 1) * 128],
                            ntrch[k2][:, :, csl],
                            start=(k2 == 0), stop=(k2 == D2 - 1),
                            perf_mode=DRow)
                    nc.vector.scalar_tensor_tensor(
                        outp[:, mt, csl], sps[:], pa_sb[:, mt:mt + 1],
                        am_sb[:, mt, csl], op0=MUL, op1=MUL)
                    nc.vector.scalar_tensor_tensor(
                        outp[:, mt, csl], ed_sb[:, mt, csl],
                        pbn_sb[:, mt:mt + 1], outp[:, mt, csl],
                        op0=MUL, op1=ADD)

            if stage <= 2:
                for mt in range(RT):
                    nc.gpsimd.dma_start(out[mt * 128:(mt + 1) * 128, :],
                                        outp[:, mt, :])
                continue

            # ---------------- phase 3: impact GEMM + combine ----------------
            for nb in range(NB):
                csl = slice(nb * 512, (nb + 1) * 512)
                nch = []
                for k2 in range(D2):
                    t = stream.tile([128, 2, 512], F8,
                                    name=f"ner{rep}_{nb}_{k2}",
                                    tag=f"ner{k2}", bufs=3)
                    nc.sync.dma_start(
                        t[:], blk(ag_ne_out[nb])[:, 2 * k2:2 * k2 + 2, :])
                    nch.append(t)
                o_blk = stream.tile([128, RT, 512], BF16,
                                    name=f"o_blk{rep}_{nb}", tag="o_blk",
                                    bufs=2)
                for mt in range(RT):
                    ips = ps.tile([128, 512], F32, name=f"ips{rep}_{nb}_{mt}",
                                  tag="ps")
                    for k2 in range(D2):
                        nc.tensor.matmul(
                            ips[:],
                            neighT_own[:, 2 * k2:2 * k2 + 2,
                                       mt * 128:(mt + 1) * 128],
                            nch[k2][:],
                            start=(k2 == 0), stop=(k2 == D2 - 1),
                            perf_mode=DRow)
                    nc.vector.scalar_tensor_tensor(
                        o_blk[:, mt, :], ips[:], pgs_sb[:, mt:mt + 1],
                        outp[:, mt, csl], op0=MUL, op1=ADD)
                nc.gpsimd.dma_start(blk(out[:, csl]), o_blk[:])

    nc.compile()
    return nc


_CACHE = {}


def _get_nc(reps=1, stage=4, mock_cc=False):
    key = (reps, stage, mock_cc)
    if key not in _CACHE:
        _CACHE[key] = build(reps, stage, mock_cc)
    return _CACHE[key]


F8NP = ml_dtypes.float8_e4m3


def make_in_maps(feature, next_feature, next_action, edges, persona_t,
                 alpha, beta, gamma):
    f = np.asarray(feature, dtype=np.float32)
    nf = np.asarray(next_feature, dtype=np.float32)
    A = np.asarray(next_action, dtype=np.float32)
    diff8 = (f - nf).astype(F8NP)
    nrm = np.sqrt((nf * nf).sum(axis=1, keepdims=True))
    nrm = np.where(nrm > 0, nrm, 1.0)
    ntr8 = np.ascontiguousarray((16.0 * nf / nrm).T).astype(F8NP)
    at8_full = np.ascontiguousarray(A.T).astype(F8NP)
    am8_full = A.astype(F8NP)
    ed8_full = np.asarray(edges, dtype=np.float32).astype(F8NP)
    gmat = np.stack([np.asarray(alpha), np.asarray(beta),
                     np.asarray(gamma)], axis=1).astype(np.float32)
    ptT = np.ascontiguousarray(np.asarray(persona_t, dtype=np.float32).T)
    in_maps = []
    for c in range(NCORES):
        rs = slice(c * R, (c + 1) * R)
        in_maps.append({
            "diff8": diff8,
            "at8": np.ascontiguousarray(at8_full[:, rs]),
            "nto8": np.ascontiguousarray(ntr8[:, rs]),
            "ntr8": ntr8,
            "am8": am8_full[rs],
            "ed8": ed8_full[rs],
            "pt": np.ascontiguousarray(ptT[:, rs]),
            "gmat": gmat,
        })
    return in_maps


def kernel(feature, next_feature, next_action, edges, persona_t,
           alpha, beta, gamma):
    nc = _get_nc(1)
    in_maps = make_in_maps(feature, next_feature, next_action, edges,
                           persona_t, alpha, beta, gamma)
    res = run_bass_kernel_spmd(nc, in_maps, list(range(NCORES)))
    return np.concatenate(
        [res.results[c]["out"].astype(np.float32) for c in range(NCORES)],
        axis=0)


# revision 20
# speedup vs baseline: 1.1458x; 1.1458x over previous
"""Trainium2 Bass kernel for the gnn_message_passing reward environment.

reference:
    diff   = feature - next_feature                    # [N, D]
    neigh  = next_action @ diff                        # [N, D]
    impact = (neigh @ neigh.T) / D                     # [N, N]
    normed = row_l2_normalize(next_feature)            # [N, D]
    sim    = normed @ normed.T                         # [N, N]
    out    = persona_a * next_action * sim             # reward_sim
           - persona_b * edges                         # reward_cost
           + persona_g * impact                        # reward_impact
    (persona_x = persona_t @ x, per-row scalars)

Distribution: 1D row shard across 8 NeuronCores (512 rows each).
All three GEMMs run in fp8e4m3 with DoubleRow perf mode (2x fp8 rate at
FD=512). diff and normed.T are precomputed host-side and staged as fp8
inputs (replicated), so phase 1 (neigh.T = diff.T @ A_shard.T) and
phase 2 (sim row-shard) have no collective dependency; the single
AllGather shares the fp8 neigh.T shards for phase 3 (impact GEMM) and
overlaps phase 2 plus the NRT rank barrier. Elementwise reward combine
is fused on DVE reading straight out of PSUM; output is written bf16.
"""
import numpy as np
import ml_dtypes
from contextlib import ExitStack

import concourse.bass as bass
import concourse.tile as tile
from concourse import bacc, mybir
from concourse.bass_utils import run_bass_kernel_spmd

N = 4096          # graph nodes
D = 1024          # feature dim
NPERS = 8         # personas
NCORES = 8
R = N // NCORES   # 512 rows per core
RT = R // 128     # 4 row tiles per shard
DT = D // 128     # 8 d-tiles
K2 = N // 256     # 16 contraction k-tile PAIRS for A @ diff (DoubleRow)
D2 = DT // 2      # 4 contraction pairs over D
NB = N // 512     # 8 output column blocks

F32 = mybir.dt.float32
BF16 = mybir.dt.bfloat16
F8 = mybir.dt.float8e4
DRow = mybir.MatmulPerfMode.DoubleRow
MUL = mybir.AluOpType.mult
ADD = mybir.AluOpType.add


def build(reps: int = 1, stage: int = 4, mock_cc: bool = False):
    nc = bacc.Bacc("TRN2", target_bir_lowering=False, debug=False,
                   num_devices=NCORES)

    diff8 = nc.dram_tensor("diff8", [N, D], F8, kind="ExternalInput").ap()
    at8 = nc.dram_tensor("at8", [N, R], F8, kind="ExternalInput").ap()
    nto8 = nc.dram_tensor("nto8", [D, R], F8, kind="ExternalInput").ap()
    ntr8 = nc.dram_tensor("ntr8", [D, N], F8, kind="ExternalInput").ap()
    am8 = nc.dram_tensor("am8", [R, N], F8, kind="ExternalInput").ap()
    ed8 = nc.dram_tensor("ed8", [R, N], F8, kind="ExternalInput").ap()
    pt = nc.dram_tensor("pt", [NPERS, R], F32, kind="ExternalInput").ap()
    gmat = nc.dram_tensor("gmat", [NPERS, 3], F32, kind="ExternalInput").ap()
    out = nc.dram_tensor("out", [R, N], BF16, kind="ExternalOutput").ap()

    rgroups = [list(range(NCORES))]

    def blk(ap):
        """[T*128, M] -> [128, T, M] partition-tiled view."""
        return ap.rearrange("(a p) m -> p a m", p=128)

    with tile.TileContext(nc) as tc, ExitStack() as ctx:
        const = ctx.enter_context(tc.tile_pool(name="const", bufs=1))
        big = ctx.enter_context(tc.tile_pool(name="big", bufs=1))
        own = ctx.enter_context(tc.tile_pool(name="own", bufs=1))
        stream = ctx.enter_context(tc.tile_pool(name="stream", bufs=1))
        outp_pool = ctx.enter_context(tc.tile_pool(name="outp", bufs=1))
        ps = ctx.enter_context(tc.tile_pool(name="ps", bufs=8, space="PSUM"))
        dram = ctx.enter_context(tc.tile_pool(name="dram", bufs=1, space="DRAM"))

        pt_sb = const.tile([NPERS, R], F32)
        nc.sync.dma_start(pt_sb[:], pt[:])
        gmat_sb = const.tile([NPERS, 3], F32)
        nc.sync.dma_start(gmat_sb[:], gmat[:])

        # phase-2 SBUF-resident operands. nto is tiny and loads up front;
        # ntr interleaves with the phase-1 streams (d8 piece per k2 pair)
        # and am/ed issue after phase 1 so its streams aren't queue-blocked.
        nto_sb = big.tile([128, DT, R], F8, name="nto_sb", tag="nto")
        nc.sync.dma_start(nto_sb[:], blk(nto8))
        ntrch = [big.tile([128, 2, N], F8, name=f"ntrch{k2}", tag=f"ntrch{k2}")
                 for k2 in range(D2)]
        am_sb = big.tile([128, RT, N], F8, name="am_sb", tag="am")
        ed_sb = big.tile([128, RT, N], F8, name="ed_sb", tag="ed")

        for rep in range(reps):
            ag_ne_in = dram.tile([D, R], F8, name=f"ag_ne_in{rep}", tag="agei")
            ag_ne_out = dram.tile([NCORES, D, R], F8, addr_space="Shared",
                                  name=f"ag_ne_out{rep}", tag="ageo")

            # ---------------- phase 1: neigh.T = diff.T @ A_shard.T ----------
            g1ps = []
            for d8 in range(DT):
                t = ps.tile([128, 512], F32, name=f"g1ps{rep}_{d8}", tag="ps")
                g1ps.append(t)
            neighT_own = own.tile([128, DT, R], F8,
                                  name=f"neown{rep}", tag="neown")
            for k2 in range(K2):
                dch = stream.tile([128, 2, D], F8, name=f"dch{rep}_{k2}",
                                  tag="dch", bufs=6)
                nc.sync.dma_start(dch[:], blk(diff8)[:, 2 * k2:2 * k2 + 2, :])
                ach = stream.tile([128, 2, R], F8, name=f"ach{rep}_{k2}",
                                  tag="ach", bufs=6)
                nc.sync.dma_start(ach[:], blk(at8)[:, 2 * k2:2 * k2 + 2, :])
                if rep == 0 and k2 % 4 == 3:
                    kp = k2 // 4
                    nc.sync.dma_start(ntrch[kp][:],
                                      blk(ntr8)[:, 2 * kp:2 * kp + 2, :])
                if rep == 0 and k2 % 4 == 1:
                    mp = k2 // 4
                    nc.sync.dma_start(am_sb[:, mp, :], blk(am8)[:, mp, :])
                if k2 < K2 - 1:
                    for d8 in range(DT):
                        nc.tensor.matmul(
                            g1ps[d8][:],
                            dch[:, :, d8 * 128:(d8 + 1) * 128],
                            ach[:], start=(k2 == 0), stop=False,
                            perf_mode=DRow)
                else:
                    # finish banks one at a time; drain (Scalar+Vector
                    # alternating) + AG-input write pipeline under the
                    # remaining MMs
                    for d8 in range(DT):
                        nc.tensor.matmul(
                            g1ps[d8][:],
                            dch[:, :, d8 * 128:(d8 + 1) * 128],
                            ach[:], start=False, stop=True,
                            perf_mode=DRow)
                        if d8 % 2 == 0:
                            nc.scalar.copy(neighT_own[:, d8, :], g1ps[d8][:])
                        else:
                            nc.vector.tensor_copy(neighT_own[:, d8, :],
                                                  g1ps[d8][:])
                        nc.sync.dma_start(
                            ag_ne_in[d8 * 128:(d8 + 1) * 128, :],
                            neighT_own[:, d8, :])

            if mock_cc:
                nc.sync.dma_start(ag_ne_out[0][:], ag_ne_in[:])
            else:
                nc.gpsimd.collective_compute(
                    "AllGather", mybir.AluOpType.bypass, ins=[ag_ne_in.opt()],
                    outs=[ag_ne_out.opt()], replica_groups=rgroups)

            if rep == 0:
                nc.sync.dma_start(ed_sb[:], blk(ed8))

            # ---------------- persona scalars (PE idle slot after phase 1) --
            # pa=alpha-mix/256 (16x-scaled normed), pbn=-beta-mix, pgs=gamma-mix/D
            pa_sb = const.tile([128, RT], F32, name=f"pa_sb{rep}", tag="pa")
            pbn_sb = const.tile([128, RT], F32, name=f"pbn_sb{rep}", tag="pbn")
            pgs_sb = const.tile([128, RT], F32, name=f"pgs_sb{rep}", tag="pgs")
            for mt in range(RT):
                pp = ps.tile([128, 512], F32, name=f"pp{rep}_{mt}", tag="ps")
                nc.tensor.matmul(pp[:, 0:3], pt_sb[:, mt * 128:(mt + 1) * 128],
                                 gmat_sb[:], start=True, stop=True)
                nc.scalar.mul(pa_sb[:, mt:mt + 1], pp[:, 0:1], 1.0 / 256)
                nc.scalar.mul(pbn_sb[:, mt:mt + 1], pp[:, 1:2], -1.0)
                nc.scalar.mul(pgs_sb[:, mt:mt + 1], pp[:, 2:3], 1.0 / D)

            if stage <= 1:
                for d8 in range(DT):
                    nc.gpsimd.dma_start(out[0:128, d8 * 512:(d8 + 1) * 512],
                                        neighT_own[:, d8, :])
                continue

            # ---------------- phase 2: sim GEMM + mask*alpha - edges*beta ----
            outp = outp_pool.tile([128, RT, N], BF16, name=f"outp{rep}",
                                  tag="outp")
            for nb in range(NB):
                csl = slice(nb * 512, (nb + 1) * 512)
                for mt in range(RT):
                    sps = ps.tile([128, 512], F32, name=f"sps{rep}_{nb}_{mt}",
                                  tag="ps")
                    for k2 in range(D2):
                        nc.tensor.matmul(
                            sps[:],
                            nto_sb[:, 2 * k2:2 * k2 + 2,
                                   mt * 128:(mt + 1) * 128],
                            ntrch[k2][:, :, csl],
                            start=(k2 == 0), stop=(k2 == D2 - 1),
                            perf_mode=DRow)
                    nc.vector.scalar_tensor_tensor(
                        outp[:, mt, csl], sps[:], pa_sb[:, mt:mt + 1],
                        am_sb[:, mt, csl], op0=MUL, op1=MUL)
                    nc.vector.scalar_tensor_tensor(
                        outp[:, mt, csl], ed_sb[:, mt, csl],
                        pbn_sb[:, mt:mt + 1], outp[:, mt, csl],
                        op0=MUL, op1=ADD)

            if stage <= 2:
                for mt in range(RT):
                    nc.gpsimd.dma_start(out[mt * 128:(mt + 1) * 128, :],
                                        outp[:, mt, :])
                continue

            # ---------------- phase 3: impact GEMM + combine ----------------
            for nb in range(NB):
                csl = slice(nb * 512, (nb + 1) * 512)
                nch = []
                for k2 in range(D2):
                    t = stream.tile([128, 2, 512], F8,
                                    name=f"ner{rep}_{nb}_{k2}",
                                    tag=f"ner{k2}", bufs=3)
                    nc.sync.dma_start(
                        t[:], blk(ag_ne_out[nb])[:, 2 * k2:2 * k2 + 2, :])
                    nch.append(t)
                o_blk = stream.tile([128, RT, 512], BF16,
                                    name=f"o_blk{rep}_{nb}", tag="o_blk",
                                    bufs=2)
                for mt in range(RT):
                    ips = ps.tile([128, 512], F32, name=f"ips{rep}_{nb}_{mt}",
                                  tag="ps")
                    for k2 in range(D2):
                        nc.tensor.matmul(
                            ips[:],
                            neighT_own[:, 2 * k2:2 * k2 + 2,
                                       mt * 128:(mt + 1) * 128],
                            nch[k2][:],
                            start=(k2 == 0), stop=(k2 == D2 - 1),
                            perf_mode=DRow)
                    nc.vector.scalar_tensor_tensor(
                        o_blk[:, mt, :], ips[:], pgs_sb[:, mt:mt + 1],
                        outp[:, mt, csl], op0=MUL, op1=ADD)
                nc.gpsimd.dma_start(blk(out[:, csl]), o_blk[:])

    nc.compile()
    return nc


_CACHE = {}


def _get_nc(reps=1, stage=4, mock_cc=False):
    key = (reps, stage, mock_cc)
    if key not in _CACHE:
        _CACHE[key] = build(reps, stage, mock_cc)
    return _CACHE[key]


F8NP = ml_dtypes.float8_e4m3


def make_in_maps(feature, next_feature, next_action, edges, persona_t,
                 alpha, beta, gamma):
    f = np.asarray(feature, dtype=np.float32)
    nf = np.asarray(next_feature, dtype=np.float32)
    A = np.asarray(next_action, dtype=np.float32)
    diff8 = (f - nf).astype(F8NP)
    nrm = np.sqrt((nf * nf).sum(axis=1, keepdims=True))
    nrm = np.where(nrm > 0, nrm, 1.0)
    ntr8 = np.ascontiguousarray((16.0 * nf / nrm).T).astype(F8NP)
    at8_full = np.ascontiguousarray(A.T).astype(F8NP)
    am8_full = A.astype(F8NP)
    ed8_full = np.asarray(edges, dtype=np.float32).astype(F8NP)
    gmat = np.stack([np.asarray(alpha), np.asarray(beta),
                     np.asarray(gamma)], axis=1).astype(np.float32)
    ptT = np.ascontiguousarray(np.asarray(persona_t, dtype=np.float32).T)
    in_maps = []
    for c in range(NCORES):
        rs = slice(c * R, (c + 1) * R)
        in_maps.append({
            "diff8": diff8,
            "at8": np.ascontiguousarray(at8_full[:, rs]),
            "nto8": np.ascontiguousarray(ntr8[:, rs]),
            "ntr8": ntr8,
            "am8": am8_full[rs],
            "ed8": ed8_full[rs],
            "pt": np.ascontiguousarray(ptT[:, rs]),
            "gmat": gmat,
        })
    return in_maps


def kernel(feature, next_feature, next_action, edges, persona_t,
           alpha, beta, gamma):
    nc = _get_nc(1)
    in_maps = make_in_maps(feature, next_feature, next_action, edges,
                           persona_t, alpha, beta, gamma)
    res = run_bass_kernel_spmd(nc, in_maps, list(range(NCORES)))
    return np.concatenate(
        [res.results[c]["out"].astype(np.float32) for c in range(NCORES)],
        axis=0)
